# revision 1
# baseline (speedup 1.0000x reference)
"""Trainium2 Bass kernel for the vq_codebook problem.

  dist_sq[n,k] = sum_d (x[n,d]-ctrs[k,d])^2 * s[d]
  out = softmax(-dist_sq, axis=1) @ values

Sharding: data-parallel over N (8192 rows of x per core); ctrs/values/s
replicated on all 8 cores. No collectives (forward only).

Math trick: softmax is shift-invariant, so
  softmax(-dist_sq)[n,k] = softmax(2*cross_s[n,k] - c_sq[k])  with
  cross_s = (x*s) @ ctrs.T,  c_sq[k] = sum_d s[d]*ctrs[k,d]^2.
We compute E = exp(2*(cross_s - 0.5*c_sq)) unnormalized (range-checked:
max exponent ~48 < 88, row-max min ~ -27, so fp32 exp never overflows
and denominators stay normal), then
  y[n,:] = (E.T @ values_aug)[n,:256] / (E.T @ values_aug)[n,256]
with values_aug = [values | ones] so the denominator comes from the same
accumulating matmul.

Layouts: phase 1 runs transposed (k on partitions, n on free) with an
augmented stationary matrix lhs1 = [[s*ctrs^T], [-0.5*c_sq]] so a single
matmul per 128-centroid chunk produces the whole softmax argument; x
tiles are transposed on the PE. Phase 2 uses E chunks as the stationary
operand against values_aug, producing y in natural [n, d_out] layout.
"""

import os

os.environ.setdefault("JAX_PLATFORMS", "axon")

import numpy as np

N, D_IN, K, D_OUT = 65536, 64, 1024, 256
NCORES = 8
NS = N // NCORES  # 8192 rows per core
TROWS = 512  # rows of x per tile
NTILES = NS // TROWS  # 16
KC = K // 128  # 8 centroid chunks
NSUB = TROWS // 128  # 4 output sub-tiles per tile

USE_F32R = True

_cache = {}


def _build(use_f32r, rows=NS, dma="sync", ph2_bf16=True):
    import concourse.bacc as bacc
    import concourse.tile as tile
    from concourse import masks, mybir

    f32 = mybir.dt.float32
    # Tiles feeding fp32r matmuls must be *written* as float32r (the engine
    # rounds on write; the BIR verifier enforces it), so the operand tiles
    # are allocated with the matmul dtype rather than bitcast at use.
    mmdt = mybir.dt.float32r if use_f32r else f32
    # Phase-2 operands in bf16: E is written by the exp activation and
    # values by a one-time copy, so both get rounded on write; bf16
    # stationary weights get fast-weight-load on the PE.
    p2dt = mybir.dt.bfloat16 if ph2_bf16 else mmdt
    Exp = mybir.ActivationFunctionType.Exp
    Copy = mybir.ActivationFunctionType.Copy

    ntiles = rows // TROWS
    nc = bacc.Bacc("TRN2", target_bir_lowering=False, debug=False)
    dma_start = {"sync": nc.sync.dma_start, "gpsimd": nc.gpsimd.dma_start}[dma]
    x = nc.declare_dram_parameter("x", [rows, D_IN], f32, isOutput=False)
    ctrs = nc.declare_dram_parameter("ctrs", [K, D_IN], f32, isOutput=False)
    values = nc.declare_dram_parameter("values", [K, D_OUT], f32, isOutput=False)
    s = nc.declare_dram_parameter("s", [D_IN], f32, isOutput=False)
    y = nc.declare_dram_parameter("y", [rows, D_OUT], f32, isOutput=True)

    with tile.TileContext(nc) as tc:
        with (
            tc.tile_pool(name="const", bufs=1) as constp,
            tc.tile_pool(name="tmp1", bufs=2) as tmp1p,
            tc.tile_pool(name="xt", bufs=4) as xtp,
            tc.tile_pool(name="xsT", bufs=3) as xsTp,
            tc.tile_pool(name="E", bufs=3) as Ep,
            tc.tile_pool(name="ysb", bufs=3) as yp,
            tc.tile_pool(name="rcp", bufs=8) as rcpp,
            tc.tile_pool(name="psA", bufs=2, space="PSUM") as psA,
            tc.tile_pool(name="psX", bufs=2, space="PSUM") as psX,
            tc.tile_pool(name="psO", bufs=2, space="PSUM") as psO,
        ):
            # -------- startup-critical prefetch: tile-0 x DMA first --------
            ident = constp.tile([128, 128], f32)
            masks.make_identity(nc, ident[:])
            ones_row = constp.tile([1, TROWS], f32)
            nc.vector.memset(ones_row[:], 1.0)

            def phase1_load(i):
                n0 = i * TROWS
                xt = xtp.tile([128, NSUB, D_IN], f32)
                dma_start(
                    xt[:], x[n0 : n0 + TROWS, :].rearrange("(a p) d -> p a d", p=128)
                )
                xsT = xsTp.tile([D_IN + 1, TROWS], mmdt)
                for p in range(NSUB // 2):
                    # Paired transpose: [128, 2, 64] -> [128, 128] PSUM with
                    # x_{2p}^T on partitions 0..63 and x_{2p+1}^T on 64..127.
                    xp = psX.tile([128, 128], f32, tag="psX")
                    nc.tensor.transpose(
                        xp[:],
                        xt[:, 2 * p : 2 * p + 2, :].rearrange("q a d -> q (a d)"),
                        ident[:],
                    )
                    c0 = 2 * p * 128
                    nc.vector.tensor_copy(xsT[0:D_IN, c0 : c0 + 128], xp[0:64, :])
                    # Upper half shifts partitions 64..127 -> 0..63; the
                    # engine write crossbar supports a shifted output base
                    # (same mechanism as the lhs1 c_sq row write below).
                    nc.vector.tensor_copy(
                        xsT[0:D_IN, c0 + 128 : c0 + 256], xp[64:128, :]
                    )
                nc.vector.tensor_copy(xsT[D_IN : D_IN + 1, :], ones_row[:])
                return xsT

            xsT0 = phase1_load(0)

            # ---------------- constants ----------------
            s_col = constp.tile([D_IN, 1], f32)
            dma_start(s_col[:], s[:].rearrange("(p o) -> p o", o=1))
            ctrs_nat = constp.tile([128, KC, D_IN], f32)
            dma_start(
                ctrs_nat[:], ctrs[:].rearrange("(c p) d -> p c d", p=128)
            )

            # lhs1[0:64, c, :]  = s[d] * ctrs^T chunk      (d on partitions)
            # lhs1[64, c, :]    = -0.5 * c_sq chunk        (k on free)
            lhs1 = constp.tile([D_IN + 1, KC, 128], mmdt)
            for c in range(KC):
                tp = psX.tile([D_IN, TROWS], f32, tag="psX")
                nc.tensor.transpose(tp[:, 0:128], ctrs_nat[:, c, :], ident[:])
                nc.scalar.activation(
                    lhs1[0:D_IN, c, :], tp[:, 0:128], Copy, scale=s_col[:]
                )
                tmp = tmp1p.tile([D_IN, 128], f32)
                nc.scalar.square(tmp[:], tp[:, 0:128])
                csq = psO.tile([1, D_OUT + 2], f32, tag="psO")
                # csq[0, k] = sum_d s[d] * ctrs[k, d]^2   (s_col as stationary)
                nc.tensor.matmul(csq[0:1, 0:128], s_col[:], tmp[:])
                nc.scalar.activation(
                    lhs1[D_IN : D_IN + 1, c, :], csq[0:1, 0:128], Copy, scale=-0.5
                )

            # values staging is only needed once phase 2 of tile 0 starts
            vals_stage = constp.tile([128, KC, D_OUT], f32)
            dma_start(
                vals_stage[:], values[:].rearrange("(c p) v -> p c v", p=128)
            )
            ones_kc = constp.tile([128, KC, 2], f32)
            nc.vector.memset(ones_kc[:], 1.0)
            vals = constp.tile([128, KC, D_OUT + 2], p2dt)
            nc.vector.tensor_copy(vals[:, :, 0:D_OUT], vals_stage[:])
            nc.vector.tensor_copy(vals[:, :, D_OUT : D_OUT + 2], ones_kc[:])

            # ---------------- main loop ----------------
            def phase1_mm(xsT):
                E = Ep.tile([128, KC, TROWS], p2dt)
                for c in range(0, KC, 2):
                    pe = psA.tile([128, 2, TROWS], f32, tag="psA")
                    nc.tensor.matmul(pe[:, 0, :], lhs1[:, c, :], xsT[:])
                    nc.tensor.matmul(pe[:, 1, :], lhs1[:, c + 1, :], xsT[:])
                    nc.scalar.activation(E[:, c : c + 2, :], pe[:], Exp, scale=2.0)
                return E

            def phase2(i, E):
                n0 = i * TROWS
                ysb = yp.tile([128, NSUB, D_OUT], f32)
                for a in range(NSUB):
                    po = psO.tile([128, D_OUT + 2], f32, tag="psO")
                    for c in range(KC):
                        nc.tensor.matmul(
                            po[:],
                            E[:, c, a * 128 : (a + 1) * 128],
                            vals[:, c, :],
                            start=(c == 0),
                            stop=(c == KC - 1),
                        )
                    rcp = rcpp.tile([128, 1], f32)
                    nc.vector.reciprocal(rcp[:], po[:, D_OUT : D_OUT + 1])
                    nc.vector.tensor_scalar_mul(ysb[:, a, :], po[:, 0:D_OUT], rcp[:])
                dma_start(
                    y[n0 : n0 + TROWS, :].rearrange("(a p) v -> p a v", p=128),
                    ysb[:],
                )

            Eprev = None
            for i in range(ntiles):
                xsT = xsT0 if i == 0 else phase1_load(i)
                Ecur = phase1_mm(xsT)
                if Eprev is not None:
                    phase2(i - 1, Eprev)
                Eprev = Ecur
            phase2(ntiles - 1, Eprev)

    nc.compile()
    nc.finalize()
    return nc


def get_nc(use_f32r=USE_F32R, rows=NS, dma="sync", ph2_bf16=True):
    key = ("nc", use_f32r, rows, dma, ph2_bf16)
    if key not in _cache:
        _cache[key] = _build(use_f32r, rows, dma, ph2_bf16)
    return _cache[key]


def make_in_maps(x, ctrs, values, s):
    x = np.ascontiguousarray(x, dtype=np.float32)
    ctrs = np.ascontiguousarray(ctrs, dtype=np.float32)
    values = np.ascontiguousarray(values, dtype=np.float32)
    s = np.ascontiguousarray(s, dtype=np.float32)
    return [
        {
            "x": x[i * NS : (i + 1) * NS],
            "ctrs": ctrs,
            "values": values,
            "s": s,
        }
        for i in range(NCORES)
    ]


def run(x, ctrs, values, s, trace=False, use_f32r=USE_F32R, tmpdir=None):
    from concourse.bass_utils import run_bass_kernel_spmd

    nc = get_nc(use_f32r)
    res = run_bass_kernel_spmd(
        nc,
        make_in_maps(x, ctrs, values, s),
        list(range(NCORES)),
        trace=trace,
        tmpdir=tmpdir,
    )
    out = np.concatenate([res.results[i]["y"] for i in range(NCORES)], axis=0)
    return out, res


def kernel(x, ctrs, values, s):
    out, _ = run(x, ctrs, values, s, trace=False)
    return out.astype(np.float32)



# revision 4
# speedup vs baseline: 1.0324x; 1.0324x over previous
"""Trainium2 Bass kernel for the vq_codebook problem.

  dist_sq[n,k] = sum_d (x[n,d]-ctrs[k,d])^2 * s[d]
  out = softmax(-dist_sq, axis=1) @ values

Sharding: data-parallel over N (8192 rows of x per core); ctrs/values/s
replicated on all 8 cores. No collectives (forward only).

Math trick: softmax is shift-invariant, so
  softmax(-dist_sq)[n,k] = softmax(2*cross_s[n,k] - c_sq[k])  with
  cross_s = (x*s) @ ctrs.T,  c_sq[k] = sum_d s[d]*ctrs[k,d]^2.
We compute E = exp(2*(cross_s - 0.5*c_sq)) unnormalized (range-checked:
max exponent ~48 < 88, row-max min ~ -27, so fp32 exp never overflows
and denominators stay normal), then
  y[n,:] = (E.T @ values_aug)[n,:256] / (E.T @ values_aug)[n,256]
with values_aug = [values | ones] so the denominator comes from the same
accumulating matmul.

Layouts: phase 1 runs transposed (k on partitions, n on free) with an
augmented stationary matrix lhs1 = [[s*ctrs^T], [-0.5*c_sq]] so a single
matmul per 128-centroid chunk produces the whole softmax argument; x
tiles are transposed on the PE. Phase 2 uses E chunks as the stationary
operand against values_aug, producing y in natural [n, d_out] layout.
"""

import os

os.environ.setdefault("JAX_PLATFORMS", "axon")

import numpy as np

N, D_IN, K, D_OUT = 65536, 64, 1024, 256
NCORES = 8
NS = N // NCORES  # 8192 rows per core
TROWS = 512  # rows of x per tile
NTILES = NS // TROWS  # 16
KC = K // 128  # 8 centroid chunks
NSUB = TROWS // 128  # 4 output sub-tiles per tile

USE_F32R = True

_cache = {}


def _build(use_f32r, rows=NS, dma="sync", ph2_bf16=True):
    import concourse.bacc as bacc
    import concourse.tile as tile
    from concourse import masks, mybir

    f32 = mybir.dt.float32
    # Tiles feeding fp32r matmuls must be *written* as float32r (the engine
    # rounds on write; the BIR verifier enforces it), so the operand tiles
    # are allocated with the matmul dtype rather than bitcast at use.
    mmdt = mybir.dt.float32r if use_f32r else f32
    # Phase-2 operands in bf16: E is written by the exp activation and
    # values by a one-time copy, so both get rounded on write; bf16
    # stationary weights get fast-weight-load on the PE.
    p2dt = mybir.dt.bfloat16 if ph2_bf16 else mmdt
    Exp = mybir.ActivationFunctionType.Exp
    Copy = mybir.ActivationFunctionType.Copy

    ntiles = rows // TROWS
    nc = bacc.Bacc("TRN2", target_bir_lowering=False, debug=False)
    dma_start = {"sync": nc.sync.dma_start, "gpsimd": nc.gpsimd.dma_start}[dma]
    x = nc.declare_dram_parameter("x", [rows, D_IN], f32, isOutput=False)
    ctrs = nc.declare_dram_parameter("ctrs", [K, D_IN], f32, isOutput=False)
    values = nc.declare_dram_parameter("values", [K, D_OUT], f32, isOutput=False)
    s = nc.declare_dram_parameter("s", [D_IN], f32, isOutput=False)
    y = nc.declare_dram_parameter("y", [rows, D_OUT], f32, isOutput=True)

    with tile.TileContext(nc) as tc:
        with (
            tc.tile_pool(name="const", bufs=1) as constp,
            tc.tile_pool(name="tmp1", bufs=2) as tmp1p,
            tc.tile_pool(name="xt", bufs=4) as xtp,
            tc.tile_pool(name="xsT", bufs=3) as xsTp,
            tc.tile_pool(name="E", bufs=3) as Ep,
            tc.tile_pool(name="ysb", bufs=3) as yp,
            tc.tile_pool(name="rcp", bufs=8) as rcpp,
            tc.tile_pool(name="psA", bufs=2, space="PSUM") as psA,
            tc.tile_pool(name="psX", bufs=2, space="PSUM") as psX,
            tc.tile_pool(name="psO", bufs=2, space="PSUM") as psO,
        ):
            # -------- startup-critical DMAs first: ctrs gates the lhs1
            # build chain (the longest pre-loop dependency), then tile-0 x.
            # All loads use partition-contiguous layouts (row permutations)
            # so each partition line is one large descriptor.
            ctrs_nat = constp.tile([128, KC, D_IN], f32)
            dma_start(ctrs_nat[:], ctrs[:].rearrange("(p c) d -> p c d", p=128))
            s_col = constp.tile([D_IN, 1], f32)
            dma_start(s_col[:], s[:].rearrange("(p o) -> p o", o=1))

            ident = constp.tile([128, 128], f32)
            masks.make_identity(nc, ident[:])
            ones_row = constp.tile([1, TROWS], f32)
            nc.vector.memset(ones_row[:], 1.0)

            def phase1_load(i):
                n0 = i * TROWS
                xt = xtp.tile([128, NSUB, D_IN], f32)
                dma_start(
                    xt[:], x[n0 : n0 + TROWS, :].rearrange("(p a) d -> p a d", p=128)
                )
                xsT = xsTp.tile([D_IN + 1, TROWS], mmdt)
                for p in range(NSUB // 2):
                    # Paired transpose: [128, 2, 64] -> [128, 128] PSUM with
                    # x_{.,2p}^T on partitions 0..63 and x_{.,2p+1}^T on 64..127.
                    xp = psX.tile([128, 128], f32, tag="psX")
                    nc.tensor.transpose(
                        xp[:],
                        xt[:, 2 * p : 2 * p + 2, :].rearrange("q a d -> q (a d)"),
                        ident[:],
                    )
                    c0 = 2 * p * 128
                    nc.vector.tensor_copy(xsT[0:D_IN, c0 : c0 + 128], xp[0:64, :])
                    # Upper half shifts partitions 64..127 -> 0..63; the
                    # engine write crossbar supports a shifted output base
                    # (same mechanism as the lhs1 c_sq row write below).
                    nc.vector.tensor_copy(
                        xsT[0:D_IN, c0 + 128 : c0 + 256], xp[64:128, :]
                    )
                nc.vector.tensor_copy(xsT[D_IN : D_IN + 1, :], ones_row[:])
                return xsT

            xsT0 = phase1_load(0)

            # lhs1[0:64, c, :]  = s[d] * ctrs^T chunk      (d on partitions)
            # lhs1[64, c, :]    = -0.5 * c_sq chunk        (k on free)
            lhs1 = constp.tile([D_IN + 1, KC, 128], mmdt)
            for c in range(KC):
                tp = psX.tile([D_IN, TROWS], f32, tag="psX")
                nc.tensor.transpose(tp[:, 0:128], ctrs_nat[:, c, :], ident[:])
                nc.scalar.activation(
                    lhs1[0:D_IN, c, :], tp[:, 0:128], Copy, scale=s_col[:]
                )
                tmp = tmp1p.tile([D_IN, 128], f32)
                nc.scalar.square(tmp[:], tp[:, 0:128])
                csq = psO.tile([1, D_OUT + 2], f32, tag="psO")
                # csq[0, k] = sum_d s[d] * ctrs[k, d]^2   (s_col as stationary)
                nc.tensor.matmul(csq[0:1, 0:128], s_col[:], tmp[:])
                nc.scalar.activation(
                    lhs1[D_IN : D_IN + 1, c, :], csq[0:1, 0:128], Copy, scale=-0.5
                )

            # values staging is only needed once phase 2 of tile 0 starts
            vals_stage = constp.tile([128, KC, D_OUT], f32)
            dma_start(
                vals_stage[:], values[:].rearrange("(p c) v -> p c v", p=128)
            )
            ones_kc = constp.tile([128, KC, 2], f32)
            nc.vector.memset(ones_kc[:], 1.0)
            vals = constp.tile([128, KC, D_OUT + 2], p2dt)
            nc.vector.tensor_copy(vals[:, :, 0:D_OUT], vals_stage[:])
            nc.vector.tensor_copy(vals[:, :, D_OUT : D_OUT + 2], ones_kc[:])

            # ---------------- main loop ----------------
            def phase1_mm(xsT):
                E = Ep.tile([128, KC, TROWS], p2dt)
                for c in range(0, KC, 2):
                    pe = psA.tile([128, 2, TROWS], f32, tag="psA")
                    nc.tensor.matmul(pe[:, 0, :], lhs1[:, c, :], xsT[:])
                    nc.tensor.matmul(pe[:, 1, :], lhs1[:, c + 1, :], xsT[:])
                    nc.scalar.activation(E[:, c : c + 2, :], pe[:], Exp, scale=2.0)
                return E

            def phase2(i, E):
                n0 = i * TROWS
                ysb = yp.tile([128, NSUB, D_OUT], f32)
                for a in range(NSUB):
                    po = psO.tile([128, D_OUT + 2], f32, tag="psO")
                    for c in range(KC):
                        nc.tensor.matmul(
                            po[:],
                            E[:, c, a * 128 : (a + 1) * 128],
                            vals[:, c, :],
                            start=(c == 0),
                            stop=(c == KC - 1),
                        )
                    rcp = rcpp.tile([128, 1], f32)
                    nc.vector.reciprocal(rcp[:], po[:, D_OUT : D_OUT + 1])
                    nc.vector.tensor_scalar_mul(ysb[:, a, :], po[:, 0:D_OUT], rcp[:])
                dma_start(
                    y[n0 : n0 + TROWS, :].rearrange("(p a) v -> p a v", p=128),
                    ysb[:],
                )

            Eprev = None
            for i in range(ntiles):
                xsT = xsT0 if i == 0 else phase1_load(i)
                Ecur = phase1_mm(xsT)
                if Eprev is not None:
                    phase2(i - 1, Eprev)
                Eprev = Ecur
            phase2(ntiles - 1, Eprev)

    nc.compile()
    nc.finalize()
    return nc


def get_nc(use_f32r=USE_F32R, rows=NS, dma="sync", ph2_bf16=True):
    key = ("nc", use_f32r, rows, dma, ph2_bf16)
    if key not in _cache:
        _cache[key] = _build(use_f32r, rows, dma, ph2_bf16)
    return _cache[key]


def make_in_maps(x, ctrs, values, s):
    x = np.ascontiguousarray(x, dtype=np.float32)
    ctrs = np.ascontiguousarray(ctrs, dtype=np.float32)
    values = np.ascontiguousarray(values, dtype=np.float32)
    s = np.ascontiguousarray(s, dtype=np.float32)
    return [
        {
            "x": x[i * NS : (i + 1) * NS],
            "ctrs": ctrs,
            "values": values,
            "s": s,
        }
        for i in range(NCORES)
    ]


def run(x, ctrs, values, s, trace=False, use_f32r=USE_F32R, tmpdir=None):
    from concourse.bass_utils import run_bass_kernel_spmd

    nc = get_nc(use_f32r)
    res = run_bass_kernel_spmd(
        nc,
        make_in_maps(x, ctrs, values, s),
        list(range(NCORES)),
        trace=trace,
        tmpdir=tmpdir,
    )
    out = np.concatenate([res.results[i]["y"] for i in range(NCORES)], axis=0)
    return out, res


def kernel(x, ctrs, values, s):
    out, _ = run(x, ctrs, values, s, trace=False)
    return out.astype(np.float32)



# revision 7
# speedup vs baseline: 1.0410x; 1.0083x over previous
"""Trainium2 Bass kernel for the vq_codebook problem.

  dist_sq[n,k] = sum_d (x[n,d]-ctrs[k,d])^2 * s[d]
  out = softmax(-dist_sq, axis=1) @ values

Sharding: data-parallel over N (8192 rows of x per core); ctrs/values/s
replicated on all 8 cores. No collectives (forward only).

Math trick: softmax is shift-invariant, so
  softmax(-dist_sq)[n,k] = softmax(2*cross_s[n,k] - c_sq[k])  with
  cross_s = (x*s) @ ctrs.T,  c_sq[k] = sum_d s[d]*ctrs[k,d]^2.
We compute E = exp(2*(cross_s - 0.5*c_sq)) unnormalized (range-checked:
max exponent ~48 < 88, row-max min ~ -27, so fp32 exp never overflows
and denominators stay normal), then
  y[n,:] = (E.T @ values_aug)[n,:256] / (E.T @ values_aug)[n,256]
with values_aug = [values | ones] so the denominator comes from the same
accumulating matmul.

Layouts: phase 1 runs transposed (k on partitions, n on free) with an
augmented stationary matrix lhs1 = [[s*ctrs^T], [-0.5*c_sq]] so a single
matmul per 128-centroid chunk produces the whole softmax argument; x
tiles are transposed on the PE. Phase 2 uses E chunks as the stationary
operand against values_aug, producing y in natural [n, d_out] layout.
"""

import os

os.environ.setdefault("JAX_PLATFORMS", "axon")

import numpy as np

N, D_IN, K, D_OUT = 65536, 64, 1024, 256
NCORES = 8
NS = N // NCORES  # 8192 rows per core
TROWS = 512  # rows of x per tile
NTILES = NS // TROWS  # 16
KC = K // 128  # 8 centroid chunks
NSUB = TROWS // 128  # 4 output sub-tiles per tile

USE_F32R = True

_cache = {}


def _build(use_f32r, rows=NS, dma="sync", ph2_bf16=True):
    import concourse.bacc as bacc
    import concourse.tile as tile
    from concourse import masks, mybir

    f32 = mybir.dt.float32
    # Tiles feeding fp32r matmuls must be *written* as float32r (the engine
    # rounds on write; the BIR verifier enforces it), so the operand tiles
    # are allocated with the matmul dtype rather than bitcast at use.
    mmdt = mybir.dt.float32r if use_f32r else f32
    # Phase-2 operands in bf16: E is written by the exp activation and
    # values by a one-time copy, so both get rounded on write; bf16
    # stationary weights get fast-weight-load on the PE.
    p2dt = mybir.dt.bfloat16 if ph2_bf16 else mmdt
    Exp = mybir.ActivationFunctionType.Exp
    Copy = mybir.ActivationFunctionType.Copy

    ntiles = rows // TROWS
    nc = bacc.Bacc("TRN2", target_bir_lowering=False, debug=False)
    dma_start = {"sync": nc.sync.dma_start, "gpsimd": nc.gpsimd.dma_start}[dma]
    x = nc.declare_dram_parameter("x", [rows, D_IN], f32, isOutput=False)
    ctrs = nc.declare_dram_parameter("ctrs", [K, D_IN], f32, isOutput=False)
    values = nc.declare_dram_parameter("values", [K, D_OUT], f32, isOutput=False)
    s = nc.declare_dram_parameter("s", [D_IN], f32, isOutput=False)
    y = nc.declare_dram_parameter("y", [rows, D_OUT], f32, isOutput=True)

    with tile.TileContext(nc) as tc:
        with (
            tc.tile_pool(name="const", bufs=1) as constp,
            tc.tile_pool(name="tmp1", bufs=2) as tmp1p,
            tc.tile_pool(name="xt", bufs=4) as xtp,
            tc.tile_pool(name="xsT", bufs=3) as xsTp,
            tc.tile_pool(name="E", bufs=3) as Ep,
            tc.tile_pool(name="ysb", bufs=3) as yp,
            tc.tile_pool(name="rcp", bufs=8) as rcpp,
            tc.tile_pool(name="psA", bufs=2, space="PSUM") as psA,
            tc.tile_pool(name="psX", bufs=2, space="PSUM") as psX,
            tc.tile_pool(name="psO", bufs=2, space="PSUM") as psO,
        ):
            # -------- startup-critical DMAs first: ctrs gates the lhs1
            # build chain (the longest pre-loop dependency), then tile-0 x.
            # All loads use partition-contiguous layouts (row permutations)
            # so each partition line is one large descriptor.
            ctrs_nat = constp.tile([128, KC, D_IN], f32)
            ctrs_r = ctrs[:].rearrange("(p c) d -> p c d", p=128)
            # chunk 0 lands first so the lhs1 build chain starts early
            dma_start(ctrs_nat[:, 0:1, :], ctrs_r[:, 0:1, :])
            dma_start(ctrs_nat[:, 1:KC, :], ctrs_r[:, 1:KC, :])
            s_col = constp.tile([D_IN, 1], f32)
            dma_start(s_col[:], s[:].rearrange("(p o) -> p o", o=1))

            ident = constp.tile([128, 128], f32)
            masks.make_identity(nc, ident[:])
            ones_row = constp.tile([1, TROWS], f32)
            nc.vector.memset(ones_row[:], 1.0)

            # P-state warm-up: the PE only reaches full clock after ~3us of
            # continuous execution. Spin no-op transposes on the identity
            # (no DMA dependency) while the first loads are in flight so the
            # real pipeline starts at full speed.
            for _ in range(10):
                wp = psX.tile([128, 128], f32, tag="psX")
                nc.tensor.transpose(wp[:], ident[:], ident[:])

            def phase1_load(i):
                n0 = i * TROWS
                xt = xtp.tile([128, NSUB, D_IN], f32)
                dma_start(
                    xt[:], x[n0 : n0 + TROWS, :].rearrange("(p a) d -> p a d", p=128)
                )
                xsT = xsTp.tile([D_IN + 1, TROWS], mmdt)
                for p in range(NSUB // 2):
                    # Paired transpose: [128, 2, 64] -> [128, 128] PSUM with
                    # x_{.,2p}^T on partitions 0..63 and x_{.,2p+1}^T on 64..127.
                    xp = psX.tile([128, 128], f32, tag="psX")
                    nc.tensor.transpose(
                        xp[:],
                        xt[:, 2 * p : 2 * p + 2, :].rearrange("q a d -> q (a d)"),
                        ident[:],
                    )
                    c0 = 2 * p * 128
                    nc.vector.tensor_copy(xsT[0:D_IN, c0 : c0 + 128], xp[0:64, :])
                    # Upper half shifts partitions 64..127 -> 0..63; the
                    # engine write crossbar supports a shifted output base
                    # (same mechanism as the lhs1 c_sq row write below).
                    nc.vector.tensor_copy(
                        xsT[0:D_IN, c0 + 128 : c0 + 256], xp[64:128, :]
                    )
                nc.vector.tensor_copy(xsT[D_IN : D_IN + 1, :], ones_row[:])
                return xsT

            xsT0 = phase1_load(0)

            # lhs1[0:64, c, :]  = s[d] * ctrs^T chunk      (d on partitions)
            # lhs1[64, c, :]    = -0.5 * c_sq chunk        (k on free)
            lhs1 = constp.tile([D_IN + 1, KC, 128], mmdt)
            for c in range(KC):
                tp = psX.tile([D_IN, TROWS], f32, tag="psX")
                nc.tensor.transpose(tp[:, 0:128], ctrs_nat[:, c, :], ident[:])
                nc.scalar.activation(
                    lhs1[0:D_IN, c, :], tp[:, 0:128], Copy, scale=s_col[:]
                )
                tmp = tmp1p.tile([D_IN, 128], f32)
                nc.scalar.square(tmp[:], tp[:, 0:128])
                csq = psO.tile([1, D_OUT + 2], f32, tag="psO")
                # csq[0, k] = sum_d s[d] * ctrs[k, d]^2   (s_col as stationary)
                nc.tensor.matmul(csq[0:1, 0:128], s_col[:], tmp[:])
                nc.scalar.activation(
                    lhs1[D_IN : D_IN + 1, c, :], csq[0:1, 0:128], Copy, scale=-0.5
                )

            # values staging is only needed once phase 2 of tile 0 starts
            vals_stage = constp.tile([128, KC, D_OUT], f32)
            dma_start(
                vals_stage[:], values[:].rearrange("(p c) v -> p c v", p=128)
            )
            ones_kc = constp.tile([128, KC, 2], f32)
            nc.vector.memset(ones_kc[:], 1.0)
            vals = constp.tile([128, KC, D_OUT + 2], p2dt)
            nc.vector.tensor_copy(vals[:, :, 0:D_OUT], vals_stage[:])
            nc.vector.tensor_copy(vals[:, :, D_OUT : D_OUT + 2], ones_kc[:])

            # ---------------- main loop ----------------
            def phase1_mm(xsT):
                E = Ep.tile([128, KC, TROWS], p2dt)
                for c in range(0, KC, 2):
                    pe = psA.tile([128, 2, TROWS], f32, tag="psA")
                    nc.tensor.matmul(pe[:, 0, :], lhs1[:, c, :], xsT[:])
                    nc.tensor.matmul(pe[:, 1, :], lhs1[:, c + 1, :], xsT[:])
                    nc.scalar.activation(E[:, c : c + 2, :], pe[:], Exp, scale=2.0)
                return E

            def phase2(i, E):
                n0 = i * TROWS
                y_r = y[n0 : n0 + TROWS, :].rearrange("(p a) v -> p a v", p=128)
                ysb = yp.tile([128, NSUB, D_OUT], f32)
                for a in range(NSUB):
                    po = psO.tile([128, D_OUT + 2], f32, tag="psO")
                    for c in range(KC):
                        nc.tensor.matmul(
                            po[:],
                            E[:, c, a * 128 : (a + 1) * 128],
                            vals[:, c, :],
                            start=(c == 0),
                            stop=(c == KC - 1),
                        )
                    rcp = rcpp.tile([128, 1], f32)
                    nc.vector.reciprocal(rcp[:], po[:, D_OUT : D_OUT + 1])
                    nc.vector.tensor_scalar_mul(ysb[:, a, :], po[:, 0:D_OUT], rcp[:])
                    if a % 2 == 1:
                        # store each half-tile as soon as it is normalized so
                        # the final tile's store overlaps its own compute
                        dma_start(
                            y_r[:, a - 1 : a + 1, :], ysb[:, a - 1 : a + 1, :]
                        )

            Eprev = None
            for i in range(ntiles):
                xsT = xsT0 if i == 0 else phase1_load(i)
                Ecur = phase1_mm(xsT)
                if Eprev is not None:
                    phase2(i - 1, Eprev)
                Eprev = Ecur
            phase2(ntiles - 1, Eprev)

    nc.compile()
    nc.finalize()
    return nc


def get_nc(use_f32r=USE_F32R, rows=NS, dma="sync", ph2_bf16=True):
    key = ("nc", use_f32r, rows, dma, ph2_bf16)
    if key not in _cache:
        _cache[key] = _build(use_f32r, rows, dma, ph2_bf16)
    return _cache[key]


def make_in_maps(x, ctrs, values, s):
    x = np.ascontiguousarray(x, dtype=np.float32)
    ctrs = np.ascontiguousarray(ctrs, dtype=np.float32)
    values = np.ascontiguousarray(values, dtype=np.float32)
    s = np.ascontiguousarray(s, dtype=np.float32)
    return [
        {
            "x": x[i * NS : (i + 1) * NS],
            "ctrs": ctrs,
            "values": values,
            "s": s,
        }
        for i in range(NCORES)
    ]


def run(x, ctrs, values, s, trace=False, use_f32r=USE_F32R, tmpdir=None):
    from concourse.bass_utils import run_bass_kernel_spmd

    nc = get_nc(use_f32r)
    res = run_bass_kernel_spmd(
        nc,
        make_in_maps(x, ctrs, values, s),
        list(range(NCORES)),
        trace=trace,
        tmpdir=tmpdir,
    )
    out = np.concatenate([res.results[i]["y"] for i in range(NCORES)], axis=0)
    return out, res


def kernel(x, ctrs, values, s):
    out, _ = run(x, ctrs, values, s, trace=False)
    return out.astype(np.float32)

